# revision 3
# baseline (speedup 1.0000x reference)
"""Trainium2 Bass kernel for nn_Clash_net (clash energy over atom pairs), v3.

Contract: kernel(**inputs) takes FULL (unsharded) numpy inputs as produced by
setup_inputs() and returns the FULL [6] float32 energies output.

Strategy (8 NeuronCores, SPMD):
  - Shard the atom-pairs dimension P across the 8 cores (contiguous split,
    padded with sentinel pairs whose clash contribution is exactly 0).
  - Replicate a packed per-atom table [x, y, z, r] (r = radii[atom_names])
    in DRAM on every core; per-pair endpoint records (16 B) are gathered
    on-device with GPSIMD indirect DMA in the canonical [128,1]-offset form
    (128 records per call, one per partition).
  - v3 vs baseline: the whole per-core index matrix [128, 3912] and all six
    mask planes are loaded into SBUF upfront with a handful of large DMAs;
    the 7824 gather calls are issued back-to-back with no interleaved input
    DMAs so they pipeline at SWDGE rate instead of serializing against
    per-chunk loads.
  - Per chunk: dist = sqrt(|c0-c1|^2 + eps), base = r0+r1-dist, then per
    class c: acc_c += mask_c * relu(base + tol_c).
  - Per-core partial [6] sums are returned; the host sums the 8 partials
    and scales by exp(weight[0]).
"""

import sys

sys.path.insert(0, "/opt/trn_rl_repo")

import numpy as np

import concourse.bass as bass
import concourse.bacc as bacc
import concourse.mybir as mybir
import concourse.tile as tile
from concourse.bass_utils import run_bass_kernel_spmd

F32 = mybir.dt.float32
I32 = mybir.dt.int32
U8 = mybir.dt.uint8

N_CORES = 8
EPS = 1e-12

N_ATOMS = 100000
N_PAIRS = 4000000
N_CLASS = 6

PAIRS_PER_CORE = N_PAIRS // N_CORES  # 500000
CHUNK = 489
N_CHUNKS = 8
COLS = CHUNK * N_CHUNKS  # 3912
P_PAD = 128 * COLS  # 500736 >= 500000


def build_nc(p_pad, chunk, n_chunks, ntab, num_devices=N_CORES, repeat=1):
    """repeat>1 re-runs the whole pair loop (for delta-timing); output scales."""
    assert p_pad == 128 * chunk * n_chunks
    cols = chunk * n_chunks
    nc = bacc.Bacc(
        "TRN2", target_bir_lowering=False, debug=False, num_devices=num_devices
    )
    idx0 = nc.dram_tensor("idx0", [128, cols], I32, kind="ExternalInput")
    idx1 = nc.dram_tensor("idx1", [128, cols], I32, kind="ExternalInput")
    masks = nc.dram_tensor("masks", [N_CLASS, p_pad], U8, kind="ExternalInput")
    table = nc.dram_tensor("table", [ntab, 4], F32, kind="ExternalInput")
    toll = nc.dram_tensor("toll", [128, N_CLASS], F32, kind="ExternalInput")
    outp = nc.dram_tensor("out", [1, N_CLASS], F32, kind="ExternalOutput")

    with tile.TileContext(nc) as tc:
        with (
            tc.tile_pool(name="const", bufs=1) as cpool,
            tc.tile_pool(name="work", bufs=3) as wpool,
            tc.tile_pool(name="psum", bufs=1, space="PSUM") as ppool,
        ):
            n = chunk
            tolb = cpool.tile([128, N_CLASS], F32)
            nc.sync.dma_start(out=tolb[:], in_=toll[:])
            ones = cpool.tile([128, 1], F32)
            nc.vector.memset(ones[:], 1.0)
            epsb = cpool.tile([128, 1], F32)
            nc.vector.memset(epsb[:], EPS)
            acc = cpool.tile([128, N_CLASS], F32)
            nc.vector.memset(acc[:], 0.0)

            # Upfront bulk loads: index matrices and all mask planes.
            i0 = cpool.tile([128, cols], I32)
            i1 = cpool.tile([128, cols], I32)
            nc.sync.dma_start(out=i0[:], in_=idx0[:])
            nc.sync.dma_start(out=i1[:], in_=idx1[:])
            mk = cpool.tile([128, N_CLASS, cols], U8)
            masks_t = masks[:].rearrange("c (k p q) -> c k p q", k=n_chunks, p=128)
            for c in range(N_CLASS):
                for k in range(n_chunks):
                    nc.sync.dma_start(
                        out=mk[:, c, k * n : (k + 1) * n], in_=masks_t[c, k]
                    )

            for k in [kk for _ in range(repeat) for kk in range(n_chunks)]:
                g0 = wpool.tile([128, n, 4], F32, tag="g0")
                g1 = wpool.tile([128, n, 4], F32, tag="g1")
                for j in range(n):
                    nc.gpsimd.indirect_dma_start(
                        out=g0[:, j, :],
                        out_offset=None,
                        in_=table[:],
                        in_offset=bass.IndirectOffsetOnAxis(
                            ap=i0[:, k * n + j : k * n + j + 1], axis=0
                        ),
                    )
                for j in range(n):
                    nc.gpsimd.indirect_dma_start(
                        out=g1[:, j, :],
                        out_offset=None,
                        in_=table[:],
                        in_offset=bass.IndirectOffsetOnAxis(
                            ap=i1[:, k * n + j : k * n + j + 1], axis=0
                        ),
                    )

                dx = wpool.tile([128, n], F32, tag="dx")
                dy = wpool.tile([128, n], F32, tag="dy")
                dz = wpool.tile([128, n], F32, tag="dz")
                rs = wpool.tile([128, n], F32, tag="rs")
                nc.vector.tensor_sub(out=dx[:], in0=g0[:, :, 0], in1=g1[:, :, 0])
                nc.vector.tensor_sub(out=dy[:], in0=g0[:, :, 1], in1=g1[:, :, 1])
                nc.vector.tensor_sub(out=dz[:], in0=g0[:, :, 2], in1=g1[:, :, 2])
                nc.vector.tensor_add(out=rs[:], in0=g0[:, :, 3], in1=g1[:, :, 3])

                ss = wpool.tile([128, n], F32, tag="ss")
                t2 = wpool.tile([128, n], F32, tag="t2")
                nc.vector.tensor_mul(out=ss[:], in0=dx[:], in1=dx[:])
                nc.vector.tensor_mul(out=t2[:], in0=dy[:], in1=dy[:])
                nc.vector.tensor_add(out=ss[:], in0=ss[:], in1=t2[:])
                nc.vector.tensor_mul(out=t2[:], in0=dz[:], in1=dz[:])
                nc.vector.tensor_add(out=ss[:], in0=ss[:], in1=t2[:])

                dist = wpool.tile([128, n], F32, tag="dist")
                nc.scalar.activation(
                    out=dist[:],
                    in_=ss[:],
                    func=mybir.ActivationFunctionType.Sqrt,
                    bias=epsb[:],
                )
                base = wpool.tile([128, n], F32, tag="base")
                nc.vector.tensor_sub(out=base[:], in0=rs[:], in1=dist[:])

                for c in range(N_CLASS):
                    rc = wpool.tile([128, n], F32, tag="rc")
                    nc.vector.tensor_scalar(
                        out=rc[:],
                        in0=base[:],
                        scalar1=tolb[:, c : c + 1],
                        scalar2=0.0,
                        op0=mybir.AluOpType.add,
                        op1=mybir.AluOpType.max,
                    )
                    scr = wpool.tile([128, n], F32, tag="scr")
                    nc.vector.tensor_tensor(
                        out=scr[:],
                        in0=rc[:],
                        in1=mk[:, c, k * n : (k + 1) * n],
                        op=mybir.AluOpType.mult,
                    )
                    red = wpool.tile([128, 1], F32, tag="red")
                    nc.vector.tensor_reduce(
                        out=red[:],
                        in_=scr[:],
                        axis=mybir.AxisListType.X,
                        op=mybir.AluOpType.add,
                    )
                    nc.vector.tensor_add(
                        out=acc[:, c : c + 1], in0=acc[:, c : c + 1], in1=red[:]
                    )

            psum = ppool.tile([1, N_CLASS], F32, space="PSUM")
            nc.tensor.matmul(
                out=psum[:], lhsT=ones[:], rhs=acc[:], start=True, stop=True
            )
            out6 = cpool.tile([1, N_CLASS], F32)
            nc.vector.tensor_copy(out=out6[:], in_=psum[:])
            nc.sync.dma_start(out=outp[:], in_=out6[:])

    nc.compile()
    return nc


def build_empty_nc(num_devices=N_CORES):
    """Identical-I/O program that does no work: the dispatch-cost baseline."""
    nc = bacc.Bacc(
        "TRN2", target_bir_lowering=False, debug=False, num_devices=num_devices
    )
    nc.dram_tensor("idx0", [128, COLS], I32, kind="ExternalInput")
    nc.dram_tensor("idx1", [128, COLS], I32, kind="ExternalInput")
    nc.dram_tensor("masks", [N_CLASS, P_PAD], U8, kind="ExternalInput")
    nc.dram_tensor("table", [N_ATOMS + 2, 4], F32, kind="ExternalInput")
    toll = nc.dram_tensor("toll", [128, N_CLASS], F32, kind="ExternalInput")
    outp = nc.dram_tensor("out", [1, N_CLASS], F32, kind="ExternalOutput")
    with tile.TileContext(nc) as tc:
        with tc.tile_pool(name="w", bufs=1) as wpool:
            t = wpool.tile([128, N_CLASS], F32)
            nc.sync.dma_start(out=t[:], in_=toll[:])
            nc.vector.tensor_scalar(
                out=t[:1, :],
                in0=t[:1, :],
                scalar1=0.0,
                scalar2=None,
                op0=mybir.AluOpType.mult,
            )
            nc.sync.dma_start(out=outp[:], in_=t[:1, :])
    nc.compile()
    return nc


_NC_CACHE = {}


def _get_nc(repeat=1):
    key = (P_PAD, CHUNK, N_CHUNKS, repeat)
    if key not in _NC_CACHE:
        _NC_CACHE[key] = build_nc(P_PAD, CHUNK, N_CHUNKS, N_ATOMS + 2, repeat=repeat)
    return _NC_CACHE[key]


def _get_empty_nc():
    key = "empty"
    if key not in _NC_CACHE:
        _NC_CACHE[key] = build_empty_nc()
    return _NC_CACHE[key]


def _prep_inputs(coords, radii, tollerances, weight, atom_names, atom_pairs, clash_masks):
    """Host-side shard/layout prep. Returns (in_maps, exp_weight)."""
    coords = np.asarray(coords, dtype=np.float32)
    radii = np.asarray(radii, dtype=np.float32)
    tollerances = np.asarray(tollerances, dtype=np.float32)
    atom_names = np.asarray(atom_names)
    atom_pairs = np.asarray(atom_pairs)
    clash_masks = np.asarray(clash_masks)

    ntab = N_ATOMS + 2
    table = np.empty((ntab, 4), dtype=np.float32)
    table[:N_ATOMS, :3] = coords
    table[:N_ATOMS, 3] = radii[atom_names.astype(np.int64)]
    table[N_ATOMS] = (1e6, 1e6, 1e6, 0.0)
    table[N_ATOMS + 1] = (-1e6, -1e6, -1e6, 0.0)

    pairs32 = np.ascontiguousarray(atom_pairs.astype(np.int32))
    masks8 = np.ascontiguousarray(clash_masks).view(np.uint8)
    toll2d = np.ascontiguousarray(
        np.broadcast_to(tollerances.reshape(1, N_CLASS), (128, N_CLASS))
    )

    in_maps = []
    for c in range(N_CORES):
        lo, hi = c * PAIRS_PER_CORE, (c + 1) * PAIRS_PER_CORE
        i0 = np.full(P_PAD, N_ATOMS, dtype=np.int32)
        i1 = np.full(P_PAD, N_ATOMS + 1, dtype=np.int32)
        i0[:PAIRS_PER_CORE] = pairs32[lo:hi, 0]
        i1[:PAIRS_PER_CORE] = pairs32[lo:hi, 1]
        # device wants [128, COLS] with element (p, k*CHUNK+q) = flat k,p,q
        i0m = np.ascontiguousarray(
            i0.reshape(N_CHUNKS, 128, CHUNK).transpose(1, 0, 2).reshape(128, COLS)
        )
        i1m = np.ascontiguousarray(
            i1.reshape(N_CHUNKS, 128, CHUNK).transpose(1, 0, 2).reshape(128, COLS)
        )
        m = np.zeros((N_CLASS, P_PAD), dtype=np.uint8)
        m[:, :PAIRS_PER_CORE] = masks8[:, lo:hi]
        in_maps.append(
            {"idx0": i0m, "idx1": i1m, "masks": m, "table": table, "toll": toll2d}
        )
    return in_maps, float(np.exp(np.float64(np.asarray(weight).reshape(-1)[0])))


def kernel(coords, radii, tollerances, weight, atom_names, atom_pairs, clash_masks):
    nc = _get_nc()
    in_maps, wscale = _prep_inputs(
        coords, radii, tollerances, weight, atom_names, atom_pairs, clash_masks
    )
    res = run_bass_kernel_spmd(nc, in_maps, core_ids=list(range(N_CORES)))
    total = np.zeros(N_CLASS, dtype=np.float64)
    for c in range(N_CORES):
        total += res.results[c]["out"].reshape(N_CLASS).astype(np.float64)
    return (total * wscale).astype(np.float32)
